# revision 3
# baseline (speedup 1.0000x reference)
"""ColumnGNN (4-layer GCN + pooling + MLP decoder) on 8 Trainium2 NeuronCores.

Strategy (SPMD, one NEFF on all 8 cores):
  - 64 graphs -> 8 graphs/core, graph-padded node layout (GPAD slot/graph).
  - Per GCN layer: hwd = (h @ W) * dinv per node block (PE), AllGather fp32
    node-major shards -> full table in DRAM; per 128-node block: indirect-DMA
    gather of edge messages (edges sorted by dst, 32-node windows, uniform
    T_W tiles/window), segment-sum via PE matmuls with one-hot S (lhsT=S),
    then LayerNorm epilogue on DVE/ACT; PE transpose maintains h^T.
  - Pooling: sum/mean via membership matmul, max via per-graph free-dim
    reduce (pads are exactly 0 and relu makes 0 a safe identity here).
  - Decoder + global-feature MLP per core on its 8 graphs -> [8, 32] out.
"""
import numpy as np

N, E, G = 50000, 800000, 64
F_IN, H, GF, OUT, L = 64, 128, 4, 32, 4
NCORES = 8
GPC = G // NCORES          # graphs per core
P = 128
WIN = 32                   # dst window width (nodes) per seg-matmul tile
EPS = 1e-5


def _prep(x, edge_index, batch):
    """Host-side pure index/layout preprocessing. Returns per-core arrays."""
    counts = np.bincount(batch, minlength=G)          # nodes per graph
    starts = np.concatenate([[0], np.cumsum(counts)[:-1]])
    gpad = int(np.ceil(counts.max() / WIN) * WIN)
    # NPAD multiple of 128:
    npad = GPC * gpad
    if npad % P:
        gpad += (P - npad % P + GPC - 1) // GPC
        gpad = int(np.ceil(gpad / WIN) * WIN)
        npad = GPC * gpad
    assert npad % P == 0
    nblk = npad // P

    # node -> (core, local padded index) and padded-global index
    g_of = batch                                       # graph of node (sorted)
    core_of = g_of // GPC
    rank_in_graph = np.arange(N) - starts[g_of]
    local = (g_of % GPC) * gpad + rank_in_graph        # index within core
    pg = core_of * npad + local                        # padded-global index

    deg = np.bincount(edge_index[1], minlength=N).astype(np.float32) + 1.0

    # per-core edge partitioning by dst core
    dst_core = core_of[edge_index[1]]
    src_pg_all = pg[edge_index[0]]
    dst_loc_all = local[edge_index[1]]

    # window id of each edge (within its core)
    per_core = []
    nwin = npad // WIN
    maxcnt = 0
    for c in range(NCORES):
        m = dst_core == c
        sp, dl = src_pg_all[m], dst_loc_all[m]
        w = dl // WIN
        order = np.argsort(w, kind="stable")
        sp, dl, w = sp[order], dl[order], w[order]
        wc = np.bincount(w, minlength=nwin)
        maxcnt = max(maxcnt, int(wc.max()))
        per_core.append((sp, dl, w, wc))
    t_w = int(np.ceil(maxcnt / P))                     # tiles per window
    ntile = nwin * t_w                                 # tiles per core
    epad = ntile * P

    cores = []
    for c in range(NCORES):
        sp, dl, w, wc = per_core[c]
        src_idx = np.zeros(epad, np.int32)             # padded edge src (pg)
        s_col = np.full(epad, -1, np.int32)            # S column (0..31), -1 pad
        woff = np.concatenate([[0], np.cumsum(wc)[:-1]])
        for wi in range(nwin):
            n = wc[wi]
            if n == 0:
                continue
            base = wi * t_w * P
            sl = slice(woff[wi], woff[wi] + n)
            src_idx[base:base + n] = sp[sl]
            s_col[base:base + n] = dl[sl] - wi * WIN
        # tile-major -> partition layout [128, ntile]
        idx_t = src_idx.reshape(ntile, P).T.copy()     # idx[p, t]
        sc = s_col.reshape(ntile, P).T                 # [128, ntile]
        S = np.zeros((P, ntile, WIN), np.float32)
        pp, tt = np.nonzero(sc >= 0)
        S[pp, tt, sc[pp, tt]] = 1.0
        S = S.reshape(P, ntile * WIN)

        # node-level arrays in padded local layout
        xT = np.zeros((F_IN, npad), np.float32)
        degc = np.ones((npad,), np.float32)
        msel = np.zeros((P, nblk * GPC), np.float32)   # membership lhsT blocks
        mine = np.nonzero(core_of == c)[0]
        xT[:, local[mine]] = x[mine].T
        degc[local[mine]] = deg[mine]
        lg = (batch[mine] % GPC)
        # membership: M[p, b*GPC + g] = 1 if node b*128+p belongs to graph g
        nodes_l = local[mine]
        msel[nodes_l % P, (nodes_l // P) * GPC + lg] = 1.0
        deg_sb = degc.reshape(nblk, P).T.copy()        # [128, nblk]
        cnt = counts[c * GPC:(c + 1) * GPC].astype(np.float32).reshape(GPC, 1)
        cores.append(dict(idx=idx_t, S=S, xT=xT, deg=deg_sb, M=msel, cnt=cnt))
    return gpad, npad, nblk, nwin, t_w, ntile, cores


def _build(npad, nblk, nwin, t_w, ntile, gpad):
    import concourse.bacc as bacc
    import concourse.bass as bass
    import concourse.mybir as mybir
    import concourse.tile as tile
    f32 = mybir.dt.float32
    Alu = mybir.AluOpType
    Act = mybir.ActivationFunctionType

    nc = bacc.Bacc("TRN2", target_bir_lowering=False, num_devices=NCORES)
    TPB = t_w * (P // WIN)                             # tiles per block
    # ---- inputs (per-core data, same shapes on all cores) ----
    inp = {}
    def I(name, shape):
        inp[name] = nc.dram_tensor(name, shape, f32, kind="ExternalInput")
        return inp[name]
    idx_d = nc.dram_tensor("idx", [P, ntile], mybir.dt.int32, kind="ExternalInput")
    S_d = I("S", [P, ntile * WIN])
    xT_d = I("xT", [F_IN, npad])
    deg_d = I("deg", [P, nblk])
    M_d = I("M", [P, nblk * GPC])
    cnt_d = I("cnt", [GPC, 1])
    encW_d = I("encW", [F_IN, H])
    encb_d = I("encb", [P, H]); encg_d = I("encg", [P, H]); encbe_d = I("encbe", [P, H])
    convW_d = I("convW", [H, L * H])                  # [k, l*H]
    convb_d = I("convb", [P, L * H]); cg_d = I("cg", [P, L * H]); cb_d = I("cb", [P, L * H])
    gfT_d = I("gfT", [GF, GPC])
    gpW1_d = I("gpW1", [GF, H // 2]); gpb1_d = I("gpb1", [H // 2, 1])
    gpW2_d = I("gpW2", [H // 2, H]); gpb2_d = I("gpb2", [H, 1])
    dW1_d = I("dW1", [P, 4 * 2 * H])                  # 4 k-tiles of [128, 256]
    db1_d = I("db1", [GPC, 2 * H]); dg1_d = I("dg1", [GPC, 2 * H]); dbe1_d = I("dbe1", [GPC, 2 * H])
    dW2_d = I("dW2", [P, 2 * H]); db2_d = I("db2", [GPC, H])
    dW3_d = I("dW3", [P, OUT]); db3_d = I("db3", [GPC, OUT])
    out_d = nc.dram_tensor("out", [GPC, OUT], f32, kind="ExternalOutput")


    with tile.TileContext(nc) as tc:
        with (
            tc.tile_pool(name="state", bufs=1) as st,
            tc.tile_pool(name="sbuf", bufs=3) as sb,
            tc.tile_pool(name="spsum", bufs=2, space="PSUM") as ps_hwd,
            tc.tile_pool(name="segp", bufs=2, space="PSUM") as ps_seg,
            tc.tile_pool(name="trp", bufs=2, space="PSUM") as ps_tr,
            tc.tile_pool(name="poolp", bufs=1, space="PSUM") as ps_pool,
            tc.tile_pool(name="dram", bufs=1, space="DRAM") as dr,
        ):
            # ---- persistent state ----
            h = st.tile([P, nblk, H], f32)             # node-major per block
            hT = st.tile([P, nblk, P], f32)            # feature-major per blk
            hwd = st.tile([P, nblk, H], f32)
            dinv = st.tile([P, nblk], f32)
            idx_sb = st.tile([P, ntile], mybir.dt.int32)
            ident = st.tile([P, P], f32)
            from concourse.masks import make_identity
            make_identity(nc, ident[:])
            nc.sync.dma_start(idx_sb[:], idx_d[:])

            # weights / params resident
            def load(d, shape):
                t = st.tile(shape, f32, tag="w_" + d.name)
                nc.sync.dma_start(t[:], d[:])
                return t
            encW = load(encW_d, [F_IN, H]); encb = load(encb_d, [P, H])
            encg = load(encg_d, [P, H]); encbe = load(encbe_d, [P, H])
            convW = load(convW_d, [H, L * H]); convb = load(convb_d, [P, L * H])
            cg = load(cg_d, [P, L * H]); cb = load(cb_d, [P, L * H])
            xT = load(xT_d, [F_IN, npad])
            degt = load(deg_d, [P, nblk]); Mm = load(M_d, [P, nblk * GPC])
            cntt = load(cnt_d, [GPC, 1])
            gfT = load(gfT_d, [GF, GPC]); gpW1 = load(gpW1_d, [GF, H // 2])
            gpb1 = load(gpb1_d, [H // 2, 1]); gpW2 = load(gpW2_d, [H // 2, H])
            gpb2 = load(gpb2_d, [H, 1])
            dW1 = load(dW1_d, [P, 4 * 2 * H]); db1 = load(db1_d, [GPC, 2 * H])
            dg1 = load(dg1_d, [GPC, 2 * H]); dbe1 = load(dbe1_d, [GPC, 2 * H])
            dW2 = load(dW2_d, [P, 2 * H]); db2 = load(db2_d, [GPC, H])
            dW3 = load(dW3_d, [P, OUT]); db3 = load(db3_d, [GPC, OUT])

            # dinv = 1/sqrt(deg)
            eps_t = st.tile([P, 1], f32)
            nc.vector.memset(eps_t[:], EPS)
            nc.scalar.sqrt(dinv[:], degt[:])
            nc.vector.reciprocal(dinv[:], dinv[:])


            def epilogue(b, zpsum, extra, dvb, brep, grep, berep, resid):
                """z=(zpsum[+extra])[*dinv]+brep; LN; relu; [+resid] -> h,hT"""
                z = sb.tile([P, H], f32, tag="z")
                if extra is not None:
                    nc.vector.tensor_tensor(out=z[:], in0=zpsum[:], in1=extra, op=Alu.add)
                    if dvb is not None:
                        nc.vector.tensor_scalar(out=z[:], in0=z[:], scalar1=dvb,
                                                scalar2=None, op0=Alu.mult)
                    nc.vector.tensor_tensor(out=z[:], in0=z[:], in1=brep, op=Alu.add)
                else:
                    nc.vector.tensor_tensor(out=z[:], in0=zpsum[:], in1=brep, op=Alu.add)
                nsum = sb.tile([P, 1], f32, tag="nsum")
                nc.vector.tensor_reduce(out=nsum[:], in_=z[:], axis=mybir.AxisListType.X,
                                        op=Alu.add, negate=True)
                nmu = sb.tile([P, 1], f32, tag="nmu")
                nc.scalar.mul(nmu[:], nsum[:], 1.0 / H)
                zc = sb.tile([P, H], f32, tag="zc")
                nc.scalar.activation(zc[:], z[:], Act.Identity, bias=nmu[:, :1], scale=1.0)
                sq = sb.tile([P, H], f32, tag="sq")
                ssd = sb.tile([P, 1], f32, tag="ssd")
                nc.scalar.activation(sq[:], zc[:], Act.Square, accum_out=ssd[:, :1])
                rstd = sb.tile([P, 1], f32, tag="rstd")
                nc.scalar.activation(rstd[:], ssd[:], Act.Sqrt, bias=eps_t[:, :1], scale=1.0 / H)
                nc.vector.reciprocal(rstd[:], rstd[:])
                u = sb.tile([P, H], f32, tag="u")
                nc.vector.tensor_scalar(out=u[:], in0=zc[:], scalar1=rstd[:, :1],
                                        scalar2=None, op0=Alu.mult)
                nc.vector.tensor_tensor(out=u[:], in0=u[:], in1=grep, op=Alu.mult)
                nc.vector.tensor_tensor(out=u[:], in0=u[:], in1=berep, op=Alu.add)
                nc.vector.tensor_scalar(out=u[:], in0=u[:], scalar1=0.0, scalar2=None,
                                        op0=Alu.max)
                if resid is not None:
                    nc.vector.tensor_tensor(out=h[:, b, :], in0=u[:], in1=resid, op=Alu.add)
                else:
                    nc.vector.tensor_copy(out=h[:, b, :], in_=u[:])
                tp = ps_tr.tile([P, P], f32, space="PSUM", tag="tr")
                nc.tensor.transpose(out=tp[:], in_=h[:, b, :], identity=ident[:])
                nc.scalar.copy(hT[:, b, :], tp[:])

            # ================= encoder =================
            for b in range(nblk):
                pz = ps_hwd.tile([P, H], f32, space="PSUM")
                nc.tensor.matmul(out=pz[:], lhsT=xT[:, b * P:(b + 1) * P],
                                 rhs=encW[:], start=True, stop=True)
                epilogue(b, pz, None, None, encb[:], encg[:], encbe[:], None)

            # ================= GCN layers =================
            for l in range(L):
                Wl = convW[:, l * H:(l + 1) * H]
                ag_in = dr.tile([npad, H], f32, tag=f"ag_in{l}")
                ag_out = dr.tile([NCORES * npad, H], f32, addr_space="Shared",
                                 tag=f"ag_out{l}")
                for b in range(nblk):
                    pz = ps_hwd.tile([P, H], f32, space="PSUM")
                    nc.tensor.matmul(out=pz[:], lhsT=hT[:, b, :], rhs=Wl,
                                     start=True, stop=True)
                    nc.vector.tensor_scalar(out=hwd[:, b, :], in0=pz[:],
                                            scalar1=dinv[:, b:b + 1], scalar2=None,
                                            op0=Alu.mult)
                    nc.sync.dma_start(ag_in[b * P:(b + 1) * P, :], hwd[:, b, :])
                nc.gpsimd.collective_compute(
                    "AllGather", Alu.bypass,
                    replica_groups=[list(range(NCORES))],
                    ins=[ag_in.opt()], outs=[ag_out.opt()])
                for b in range(nblk):
                    Ssb = sb.tile([P, TPB * WIN], f32, tag="Ssb")
                    nc.sync.dma_start(Ssb[:], S_d[:, b * TPB * WIN:(b + 1) * TPB * WIN])
                    msg = sb.tile([P, TPB, H], f32, tag="msg")
                    for kk in range(TPB):
                        nc.gpsimd.indirect_dma_start(
                            out=msg[:, kk, :], out_offset=None, in_=ag_out[:],
                            in_offset=bass.IndirectOffsetOnAxis(
                                ap=idx_sb[:, b * TPB + kk:b * TPB + kk + 1], axis=0))
                    acc = ps_seg.tile([P, H], f32, space="PSUM")
                    for w in range(P // WIN):
                        for t in range(t_w):
                            k = w * t_w + t
                            nc.tensor.matmul(
                                out=acc[WIN * w:WIN * (w + 1), :],
                                lhsT=Ssb[:, k * WIN:(k + 1) * WIN],
                                rhs=msg[:, k, :],
                                start=(t == 0), stop=(t == t_w - 1),
                                tile_position=(0, WIN * w))
                    epilogue(b, acc, hwd[:, b, :], dinv[:, b:b + 1],
                             convb[:, l * H:(l + 1) * H], cg[:, l * H:(l + 1) * H],
                             cb[:, l * H:(l + 1) * H], h[:, b, :])

            # ================= pooling =================
            psum_pool = ps_pool.tile([GPC, H + 1], f32, space="PSUM")
            ones = st.tile([P, 1], f32)
            nc.vector.memset(ones[:], 1.0)
            for b in range(nblk):
                nc.tensor.matmul(out=psum_pool[:, :H], lhsT=Mm[:, b * GPC:(b + 1) * GPC],
                                 rhs=h[:, b, :], start=(b == 0), stop=(b == nblk - 1),
                                 skip_group_check=True)
                nc.tensor.matmul(out=psum_pool[:, H:H + 1], lhsT=Mm[:, b * GPC:(b + 1) * GPC],
                                 rhs=ones[:], start=(b == 0), stop=(b == nblk - 1),
                                 skip_group_check=True)
            psum = sb.tile([GPC, H], f32)
            nc.vector.tensor_copy(psum[:], psum_pool[:, :H])
            cinv = sb.tile([GPC, 1], f32)
            nc.vector.reciprocal(cinv[:], cntt[:])
            pmean = sb.tile([GPC, H], f32)
            nc.vector.tensor_scalar(out=pmean[:], in0=psum[:], scalar1=cinv[:, :1],
                                    scalar2=None, op0=Alu.mult)
            # max pool: per-graph reduce over hT free ranges
            mxT = sb.tile([P, GPC], f32)
            hT_flat = hT[:].rearrange("p b f -> p (b f)")
            for g in range(GPC):
                nc.vector.tensor_reduce(
                    out=mxT[:, g:g + 1],
                    in_=hT_flat[:, g * gpad:(g + 1) * gpad],
                    axis=mybir.AxisListType.X, op=Alu.max)
            # transpose mean/sum -> [128, GPC]
            identg = ident[:GPC, :GPC]
            meanT_p = ps_tr.tile([P, P], f32, space="PSUM", tag="tr")
            nc.tensor.transpose(out=meanT_p[:H, :GPC], in_=pmean[:], identity=identg)
            meanT = sb.tile([P, GPC], f32)
            nc.scalar.copy(meanT[:H, :], meanT_p[:H, :GPC])
            sumT_p = ps_tr.tile([P, P], f32, space="PSUM", tag="tr")
            nc.tensor.transpose(out=sumT_p[:H, :GPC], in_=psum[:], identity=identg)
            sumT = sb.tile([P, GPC], f32)
            nc.scalar.copy(sumT[:H, :], sumT_p[:H, :GPC])

            # ================= global-feature MLP (feature-major) ==========
            r1p = ps_seg.tile([H // 2, GPC], f32, space="PSUM", tag="acc")
            nc.tensor.matmul(out=r1p[:], lhsT=gpW1[:], rhs=gfT[:], start=True, stop=True)
            r1 = sb.tile([H // 2, GPC], f32)
            nc.scalar.activation(r1[:], r1p[:], Act.Relu, bias=gpb1[:, :1], scale=1.0)
            gep = ps_seg.tile([H, GPC], f32, space="PSUM", tag="acc")
            nc.tensor.matmul(out=gep[:], lhsT=gpW2[:], rhs=r1[:], start=True, stop=True)
            geT = sb.tile([H, GPC], f32)
            nc.scalar.activation(geT[:], gep[:], Act.Identity, bias=gpb2[:, :1], scale=1.0)

            # ================= decoder =================
            z1p = ps_pool.tile([GPC, 2 * H], f32, space="PSUM", tag="dec")
            for i, kt in enumerate([meanT, mxT, sumT, geT]):
                nc.tensor.matmul(out=z1p[:], lhsT=kt[:, :GPC] if i != 3 else kt[:],
                                 rhs=dW1[:, i * 2 * H:(i + 1) * 2 * H],
                                 start=(i == 0), stop=(i == 3), skip_group_check=True)
            # LN(z1) graph-major [GPC, 256] + relu
            z1 = sb.tile([GPC, 2 * H], f32)
            nc.vector.tensor_tensor(out=z1[:], in0=z1p[:], in1=db1[:], op=Alu.add)
            nsum = sb.tile([GPC, 1], f32)
            nc.vector.tensor_reduce(out=nsum[:], in_=z1[:], axis=mybir.AxisListType.X,
                                    op=Alu.add, negate=True)
            nmu = sb.tile([GPC, 1], f32)
            nc.scalar.mul(nmu[:], nsum[:], 1.0 / (2 * H))
            zc = sb.tile([GPC, 2 * H], f32)
            nc.scalar.activation(zc[:], z1[:], Act.Identity, bias=nmu[:, :1], scale=1.0)
            sq = sb.tile([GPC, 2 * H], f32)
            ssd = sb.tile([GPC, 1], f32)
            nc.scalar.activation(sq[:], zc[:], Act.Square, accum_out=ssd[:, :1])
            rstd = sb.tile([GPC, 1], f32)
            nc.scalar.activation(rstd[:], ssd[:], Act.Sqrt, bias=eps_t[:GPC, :1], scale=1.0 / (2 * H))
            nc.vector.reciprocal(rstd[:], rstd[:])
            nc.vector.tensor_scalar(out=zc[:], in0=zc[:], scalar1=rstd[:, :1],
                                    scalar2=None, op0=Alu.mult)
            nc.vector.tensor_tensor(out=zc[:], in0=zc[:], in1=dg1[:], op=Alu.mult)
            nc.vector.tensor_tensor(out=zc[:], in0=zc[:], in1=dbe1[:], op=Alu.add)
            nc.vector.tensor_scalar(out=zc[:], in0=zc[:], scalar1=0.0, scalar2=None,
                                    op0=Alu.max)
            # z2 = relu(z1n @ dW2): need z1n^T k-tiles [128, GPC] x2
            z2p_t = ps_pool.tile([GPC, 2 * H], f32, space="PSUM", tag="dec")
            z2p = z2p_t[:, :H]
            for i in range(2):
                ztp = ps_tr.tile([P, P], f32, space="PSUM", tag="tr")
                nc.tensor.transpose(out=ztp[:, :GPC], in_=zc[:, i * P:(i + 1) * P],
                                    identity=identg)
                zt = sb.tile([P, GPC], f32, tag="zt")
                nc.scalar.copy(zt[:], ztp[:, :GPC])
                nc.tensor.matmul(out=z2p[:], lhsT=zt[:], rhs=dW2[:, i * H:(i + 1) * H],
                                 start=(i == 0), stop=(i == 1), skip_group_check=True)
            z2 = sb.tile([GPC, H], f32)
            nc.vector.tensor_tensor(out=z2[:], in0=z2p[:], in1=db2[:], op=Alu.add)
            nc.vector.tensor_scalar(out=z2[:], in0=z2[:], scalar1=0.0, scalar2=None,
                                    op0=Alu.max)
            # z3 = z2 @ dW3 + b3 -> softmax
            z2tp = ps_tr.tile([P, P], f32, space="PSUM", tag="tr")
            nc.tensor.transpose(out=z2tp[:, :GPC], in_=z2[:], identity=identg)
            z2t = sb.tile([P, GPC], f32)
            nc.scalar.copy(z2t[:], z2tp[:, :GPC])
            z3p_t = ps_pool.tile([GPC, 2 * H], f32, space="PSUM", tag="dec")
            z3p = z3p_t[:, :OUT]
            nc.tensor.matmul(out=z3p[:], lhsT=z2t[:], rhs=dW3[:], start=True, stop=True)
            z3 = sb.tile([GPC, OUT], f32)
            nc.vector.tensor_tensor(out=z3[:], in0=z3p[:], in1=db3[:], op=Alu.add)
            nmx = sb.tile([GPC, 1], f32)
            nc.vector.tensor_reduce(out=nmx[:], in_=z3[:], axis=mybir.AxisListType.X,
                                    op=Alu.max, negate=True)
            ez = sb.tile([GPC, OUT], f32)
            nc.scalar.activation(ez[:], z3[:], Act.Exp, bias=nmx[:, :1], scale=1.0)
            sez = sb.tile([GPC, 1], f32)
            nc.vector.tensor_reduce(out=sez[:], in_=ez[:], axis=mybir.AxisListType.X,
                                    op=Alu.add)
            nc.vector.reciprocal(sez[:], sez[:])
            res = sb.tile([GPC, OUT], f32)
            nc.vector.tensor_scalar(out=res[:], in0=ez[:], scalar1=sez[:, :1],
                                    scalar2=None, op0=Alu.mult)
            nc.sync.dma_start(out_d[:], res[:])
    nc.finalize()
    return nc


_CACHE = {}


def kernel(**inputs):
    x = np.asarray(inputs["x"], np.float32)
    edge_index = np.asarray(inputs["edge_index"], np.int32)
    batch = np.asarray(inputs["batch"], np.int32)
    gpad, npad, nblk, nwin, t_w, ntile, cores = _prep(x, edge_index, batch)

    key = (gpad, t_w)
    if key not in _CACHE:
        _CACHE[key] = _build(npad, nblk, nwin, t_w, ntile, gpad)
    nc = _CACHE[key]

    def rep(v, rows):  # replicate 1-D param across partitions
        return np.tile(np.asarray(v, np.float32)[None, :], (rows, 1))

    convW = np.asarray(inputs["conv_W"], np.float32)      # [L, H, H]
    shared = dict(
        encW=np.asarray(inputs["enc_W"], np.float32),
        encb=rep(inputs["enc_b"], P), encg=rep(inputs["enc_gamma"], P),
        encbe=rep(inputs["enc_beta"], P),
        convW=np.concatenate([convW[l] for l in range(L)], axis=1),
        convb=np.concatenate([rep(inputs["conv_b"][l], P) for l in range(L)], 1),
        cg=np.concatenate([rep(inputs["norm_gamma"][l], P) for l in range(L)], 1),
        cb=np.concatenate([rep(inputs["norm_beta"][l], P) for l in range(L)], 1),
        gpW1=np.asarray(inputs["gp_W1"], np.float32),
        gpb1=np.asarray(inputs["gp_b1"], np.float32)[:, None],
        gpW2=np.asarray(inputs["gp_W2"], np.float32),
        gpb2=np.asarray(inputs["gp_b2"], np.float32)[:, None],
        dW1=np.concatenate([np.asarray(inputs["dec_W1"], np.float32)[i * P:(i + 1) * P, :]
                            for i in range(4)], axis=1),
        db1=rep(inputs["dec_b1"], GPC), dg1=rep(inputs["dec_gamma"], GPC),
        dbe1=rep(inputs["dec_beta"], GPC),
        dW2=np.concatenate([np.asarray(inputs["dec_W2"], np.float32)[i * P:(i + 1) * P, :]
                            for i in range(2)], axis=1),
        db2=rep(inputs["dec_b2"], GPC),
        dW3=np.asarray(inputs["dec_W3"], np.float32),
        db3=rep(inputs["dec_b3"], GPC),
    )
    gfull = np.asarray(inputs["global_features"], np.float32)
    in_maps = []
    for c in range(NCORES):
        m = dict(shared)
        m.update(idx=cores[c]["idx"], S=cores[c]["S"], xT=cores[c]["xT"],
                 deg=cores[c]["deg"], M=cores[c]["M"], cnt=cores[c]["cnt"],
                 gfT=gfull[c * GPC:(c + 1) * GPC].T.copy())
        in_maps.append({k: np.ascontiguousarray(v, np.float32) if k != "idx"
                        else np.ascontiguousarray(v, np.int32) for k, v in m.items()})

    from concourse.bass_utils import run_bass_kernel_spmd
    global LAST_NC, LAST_IN_MAPS
    LAST_NC, LAST_IN_MAPS = nc, in_maps
    res = run_bass_kernel_spmd(nc, in_maps, core_ids=list(range(NCORES)))
    global LAST_EXEC_NS
    LAST_EXEC_NS = res.exec_time_ns
    out = np.concatenate([res.results[c]["out"] for c in range(NCORES)], axis=0)
    return out.astype(np.float32)



# revision 7
# speedup vs baseline: 1.0500x; 1.0500x over previous
"""ColumnGNN (4-layer GCN + pooling + MLP decoder) on 8 Trainium2 NeuronCores.

Strategy (SPMD, one NEFF on all 8 cores):
  - 64 graphs -> 8 graphs/core, graph-padded node layout (GPAD slot/graph).
  - Per GCN layer: hwd = (h @ W) * dinv per node block (PE), AllGather fp32
    node-major shards -> full table in DRAM; per 128-node block: indirect-DMA
    gather of edge messages (edges sorted by dst, 32-node windows, uniform
    T_W tiles/window), segment-sum via PE matmuls with one-hot S (lhsT=S),
    then LayerNorm epilogue on DVE/ACT; PE transpose maintains h^T.
  - Pooling: sum/mean via membership matmul, max via per-graph free-dim
    reduce (pads are exactly 0 and relu makes 0 a safe identity here).
  - Decoder + global-feature MLP per core on its 8 graphs -> [8, 32] out.
  - 4 SWDGE queues so the ~1us fixed cost of each indirect edge-gather DMA
    overlaps across queues (the measured bottleneck of this kernel).
"""
import numpy as np

N, E, G = 50000, 800000, 64
F_IN, H, GF, OUT, L = 64, 128, 4, 32, 4
NCORES = 8
GPC = G // NCORES          # graphs per core
P = 128
WIN = 32                   # dst window width (nodes) per seg-matmul tile
EPS = 1e-5

LAST_EXEC_NS = None
LAST_NC = None
LAST_IN_MAPS = None


def _prep(x, edge_index, batch):
    """Host-side pure index/layout preprocessing. Returns per-core arrays."""
    counts = np.bincount(batch, minlength=G)          # nodes per graph
    starts = np.concatenate([[0], np.cumsum(counts)[:-1]])
    gpad = int(np.ceil(counts.max() / WIN) * WIN)
    # NPAD multiple of 128:
    npad = GPC * gpad
    if npad % P:
        gpad += (P - npad % P + GPC - 1) // GPC
        gpad = int(np.ceil(gpad / WIN) * WIN)
        npad = GPC * gpad
    assert npad % P == 0
    nblk = npad // P

    # node -> (core, local padded index) and padded-global index
    g_of = batch                                       # graph of node (sorted)
    core_of = g_of // GPC
    rank_in_graph = np.arange(N) - starts[g_of]
    local = (g_of % GPC) * gpad + rank_in_graph        # index within core
    pg = core_of * npad + local                        # padded-global index

    deg = np.bincount(edge_index[1], minlength=N).astype(np.float32) + 1.0

    # per-core edge partitioning by dst core
    dst_core = core_of[edge_index[1]]
    src_pg_all = pg[edge_index[0]]
    dst_loc_all = local[edge_index[1]]

    # window id of each edge (within its core)
    per_core = []
    nwin = npad // WIN
    maxcnt = 0
    for c in range(NCORES):
        m = dst_core == c
        sp, dl = src_pg_all[m], dst_loc_all[m]
        w = dl // WIN
        order = np.argsort(w, kind="stable")
        sp, dl, w = sp[order], dl[order], w[order]
        wc = np.bincount(w, minlength=nwin)
        maxcnt = max(maxcnt, int(wc.max()))
        per_core.append((sp, dl, w, wc))
    t_w = int(np.ceil(maxcnt / P))                     # tiles per window
    ntile = nwin * t_w                                 # tiles per core
    epad = ntile * P

    cores = []
    for c in range(NCORES):
        sp, dl, w, wc = per_core[c]
        src_idx = np.zeros(epad, np.int32)             # padded edge src (pg)
        s_col = np.full(epad, -1, np.int32)            # S column (0..31), -1 pad
        woff = np.concatenate([[0], np.cumsum(wc)[:-1]])
        for wi in range(nwin):
            n = wc[wi]
            if n == 0:
                continue
            base = wi * t_w * P
            sl = slice(woff[wi], woff[wi] + n)
            src_idx[base:base + n] = sp[sl]
            s_col[base:base + n] = dl[sl] - wi * WIN
        # tile-major -> partition layout [128, ntile]
        idx_t = src_idx.reshape(ntile, P).T.copy()     # idx[p, t]
        sc = s_col.reshape(ntile, P).T                 # [128, ntile]
        S = np.zeros((P, ntile, WIN), np.float32)
        pp, tt = np.nonzero(sc >= 0)
        S[pp, tt, sc[pp, tt]] = 1.0
        S = S.reshape(P, ntile * WIN)

        # node-level arrays in padded local layout
        xT = np.zeros((F_IN, npad), np.float32)
        degc = np.ones((npad,), np.float32)
        msel = np.zeros((P, nblk * GPC), np.float32)   # membership lhsT blocks
        mine = np.nonzero(core_of == c)[0]
        xT[:, local[mine]] = x[mine].T
        degc[local[mine]] = deg[mine]
        lg = (batch[mine] % GPC)
        # membership: M[p, b*GPC + g] = 1 if node b*128+p belongs to graph g
        nodes_l = local[mine]
        msel[nodes_l % P, (nodes_l // P) * GPC + lg] = 1.0
        deg_sb = degc.reshape(nblk, P).T.copy()        # [128, nblk]
        cnt = counts[c * GPC:(c + 1) * GPC].astype(np.float32).reshape(GPC, 1)
        cores.append(dict(idx=idx_t, S=S, xT=xT, deg=deg_sb, M=msel, cnt=cnt))
    return gpad, npad, nblk, nwin, t_w, ntile, cores


def _build(npad, nblk, nwin, t_w, ntile, gpad):
    import concourse.bacc as bacc
    import concourse.bass as bass
    import concourse.mybir as mybir
    import concourse.tile as tile
    f32 = mybir.dt.float32
    Alu = mybir.AluOpType
    Act = mybir.ActivationFunctionType

    nc = bacc.Bacc("TRN2", target_bir_lowering=False, num_devices=NCORES,
                   num_swdge_queues=4)
    TPB = t_w * (P // WIN)                             # tiles per block
    # ---- inputs (per-core data, same shapes on all cores) ----
    inp = {}
    def I(name, shape):
        inp[name] = nc.dram_tensor(name, shape, f32, kind="ExternalInput")
        return inp[name]
    idx_d = nc.dram_tensor("idx", [P, ntile], mybir.dt.int32, kind="ExternalInput")
    S_d = I("S", [P, ntile * WIN])
    xT_d = I("xT", [F_IN, npad])
    deg_d = I("deg", [P, nblk])
    M_d = I("M", [P, nblk * GPC])
    cnt_d = I("cnt", [GPC, 1])
    encW_d = I("encW", [F_IN, H])
    encb_d = I("encb", [P, H]); encg_d = I("encg", [P, H]); encbe_d = I("encbe", [P, H])
    convW_d = I("convW", [H, L * H])                  # [k, l*H]
    convb_d = I("convb", [P, L * H]); cg_d = I("cg", [P, L * H]); cb_d = I("cb", [P, L * H])
    gfT_d = I("gfT", [GF, GPC])
    gpW1_d = I("gpW1", [GF, H // 2]); gpb1_d = I("gpb1", [H // 2, 1])
    gpW2_d = I("gpW2", [H // 2, H]); gpb2_d = I("gpb2", [H, 1])
    dW1_d = I("dW1", [P, 4 * 2 * H])                  # 4 k-tiles of [128, 256]
    db1_d = I("db1", [GPC, 2 * H]); dg1_d = I("dg1", [GPC, 2 * H]); dbe1_d = I("dbe1", [GPC, 2 * H])
    dW2_d = I("dW2", [P, 2 * H]); db2_d = I("db2", [GPC, H])
    dW3_d = I("dW3", [P, OUT]); db3_d = I("db3", [GPC, OUT])
    out_d = nc.dram_tensor("out", [GPC, OUT], f32, kind="ExternalOutput")


    with tile.TileContext(nc) as tc:
        with (
            tc.tile_pool(name="state", bufs=1) as st,
            tc.tile_pool(name="sbuf", bufs=3) as sb,
            tc.tile_pool(name="spsum", bufs=2, space="PSUM") as ps_hwd,
            tc.tile_pool(name="segp", bufs=2, space="PSUM") as ps_seg,
            tc.tile_pool(name="trp", bufs=2, space="PSUM") as ps_tr,
            tc.tile_pool(name="poolp", bufs=1, space="PSUM") as ps_pool,
            tc.tile_pool(name="dram", bufs=1, space="DRAM") as dr,
        ):
            # ---- persistent state ----
            h = st.tile([P, nblk, H], f32)             # node-major per block
            hT = st.tile([P, nblk, P], f32)            # feature-major per blk
            hwd = st.tile([P, nblk, H], f32)
            dinv = st.tile([P, nblk], f32)
            idx_sb = st.tile([P, ntile], mybir.dt.int32)
            ident = st.tile([P, P], f32)
            from concourse.masks import make_identity
            make_identity(nc, ident[:])
            nc.sync.dma_start(idx_sb[:], idx_d[:])

            # weights / params resident
            def load(d, shape):
                t = st.tile(shape, f32, tag="w_" + d.name)
                nc.sync.dma_start(t[:], d[:])
                return t
            encW = load(encW_d, [F_IN, H]); encb = load(encb_d, [P, H])
            encg = load(encg_d, [P, H]); encbe = load(encbe_d, [P, H])
            convW = load(convW_d, [H, L * H]); convb = load(convb_d, [P, L * H])
            cg = load(cg_d, [P, L * H]); cb = load(cb_d, [P, L * H])
            xT = load(xT_d, [F_IN, npad])
            degt = load(deg_d, [P, nblk]); Mm = load(M_d, [P, nblk * GPC])
            cntt = load(cnt_d, [GPC, 1])
            gfT = load(gfT_d, [GF, GPC]); gpW1 = load(gpW1_d, [GF, H // 2])
            gpb1 = load(gpb1_d, [H // 2, 1]); gpW2 = load(gpW2_d, [H // 2, H])
            gpb2 = load(gpb2_d, [H, 1])
            dW1 = load(dW1_d, [P, 4 * 2 * H]); db1 = load(db1_d, [GPC, 2 * H])
            dg1 = load(dg1_d, [GPC, 2 * H]); dbe1 = load(dbe1_d, [GPC, 2 * H])
            dW2 = load(dW2_d, [P, 2 * H]); db2 = load(db2_d, [GPC, H])
            dW3 = load(dW3_d, [P, OUT]); db3 = load(db3_d, [GPC, OUT])

            # dinv = 1/sqrt(deg)
            eps_t = st.tile([P, 1], f32)
            nc.vector.memset(eps_t[:], EPS)
            nc.scalar.sqrt(dinv[:], degt[:])
            nc.vector.reciprocal(dinv[:], dinv[:])


            def epilogue(b, zpsum, extra, dvb, brep, grep, berep, resid):
                """z=(zpsum[+extra])[*dinv]+brep; LN; relu; [+resid] -> h,hT"""
                z = sb.tile([P, H], f32, tag="z")
                if extra is not None:
                    nc.vector.tensor_tensor(out=z[:], in0=zpsum[:], in1=extra, op=Alu.add)
                    if dvb is not None:
                        nc.vector.tensor_scalar(out=z[:], in0=z[:], scalar1=dvb,
                                                scalar2=None, op0=Alu.mult)
                    nc.vector.tensor_tensor(out=z[:], in0=z[:], in1=brep, op=Alu.add)
                else:
                    nc.vector.tensor_tensor(out=z[:], in0=zpsum[:], in1=brep, op=Alu.add)
                nsum = sb.tile([P, 1], f32, tag="nsum")
                nc.vector.tensor_reduce(out=nsum[:], in_=z[:], axis=mybir.AxisListType.X,
                                        op=Alu.add, negate=True)
                nmu = sb.tile([P, 1], f32, tag="nmu")
                nc.scalar.mul(nmu[:], nsum[:], 1.0 / H)
                zc = sb.tile([P, H], f32, tag="zc")
                nc.scalar.activation(zc[:], z[:], Act.Identity, bias=nmu[:, :1], scale=1.0)
                sq = sb.tile([P, H], f32, tag="sq")
                ssd = sb.tile([P, 1], f32, tag="ssd")
                nc.scalar.activation(sq[:], zc[:], Act.Square, accum_out=ssd[:, :1])
                rstd = sb.tile([P, 1], f32, tag="rstd")
                nc.scalar.activation(rstd[:], ssd[:], Act.Sqrt, bias=eps_t[:, :1], scale=1.0 / H)
                nc.vector.reciprocal(rstd[:], rstd[:])
                u = sb.tile([P, H], f32, tag="u")
                nc.vector.tensor_scalar(out=u[:], in0=zc[:], scalar1=rstd[:, :1],
                                        scalar2=None, op0=Alu.mult)
                nc.vector.tensor_tensor(out=u[:], in0=u[:], in1=grep, op=Alu.mult)
                nc.vector.tensor_tensor(out=u[:], in0=u[:], in1=berep, op=Alu.add)
                nc.vector.tensor_scalar(out=u[:], in0=u[:], scalar1=0.0, scalar2=None,
                                        op0=Alu.max)
                if resid is not None:
                    nc.vector.tensor_tensor(out=h[:, b, :], in0=u[:], in1=resid, op=Alu.add)
                else:
                    nc.vector.tensor_copy(out=h[:, b, :], in_=u[:])
                tp = ps_tr.tile([P, P], f32, space="PSUM", tag="tr")
                nc.tensor.transpose(out=tp[:], in_=h[:, b, :], identity=ident[:])
                nc.scalar.copy(hT[:, b, :], tp[:])

            # ================= encoder =================
            for b in range(nblk):
                pz = ps_hwd.tile([P, H], f32, space="PSUM")
                nc.tensor.matmul(out=pz[:], lhsT=xT[:, b * P:(b + 1) * P],
                                 rhs=encW[:], start=True, stop=True)
                epilogue(b, pz, None, None, encb[:], encg[:], encbe[:], None)

            # ================= GCN layers =================
            for l in range(L):
                Wl = convW[:, l * H:(l + 1) * H]
                ag_in = dr.tile([npad, H], f32, tag=f"ag_in{l}")
                ag_out = dr.tile([NCORES * npad, H], f32, addr_space="Shared",
                                 tag=f"ag_out{l}")
                for b in range(nblk):
                    pz = ps_hwd.tile([P, H], f32, space="PSUM")
                    nc.tensor.matmul(out=pz[:], lhsT=hT[:, b, :], rhs=Wl,
                                     start=True, stop=True)
                    nc.vector.tensor_scalar(out=hwd[:, b, :], in0=pz[:],
                                            scalar1=dinv[:, b:b + 1], scalar2=None,
                                            op0=Alu.mult)
                    nc.sync.dma_start(ag_in[b * P:(b + 1) * P, :], hwd[:, b, :])
                nc.gpsimd.collective_compute(
                    "AllGather", Alu.bypass,
                    replica_groups=[list(range(NCORES))],
                    ins=[ag_in.opt()], outs=[ag_out.opt()])
                for b in range(nblk):
                    Ssb = sb.tile([P, TPB * WIN], f32, tag="Ssb")
                    nc.sync.dma_start(Ssb[:], S_d[:, b * TPB * WIN:(b + 1) * TPB * WIN])
                    msg = sb.tile([P, TPB, H], f32, tag="msg")
                    for kk in range(TPB):
                        nc.gpsimd.indirect_dma_start(
                            out=msg[:, kk, :], out_offset=None, in_=ag_out[:],
                            in_offset=bass.IndirectOffsetOnAxis(
                                ap=idx_sb[:, b * TPB + kk:b * TPB + kk + 1], axis=0))
                    acc = ps_seg.tile([P, H], f32, space="PSUM")
                    for w in range(P // WIN):
                        for t in range(t_w):
                            k = w * t_w + t
                            nc.tensor.matmul(
                                out=acc[WIN * w:WIN * (w + 1), :],
                                lhsT=Ssb[:, k * WIN:(k + 1) * WIN],
                                rhs=msg[:, k, :],
                                start=(t == 0), stop=(t == t_w - 1),
                                tile_position=(0, WIN * w))
                    epilogue(b, acc, hwd[:, b, :], dinv[:, b:b + 1],
                             convb[:, l * H:(l + 1) * H], cg[:, l * H:(l + 1) * H],
                             cb[:, l * H:(l + 1) * H], h[:, b, :])

            # ================= pooling =================
            psum_pool = ps_pool.tile([GPC, H + 1], f32, space="PSUM")
            ones = st.tile([P, 1], f32)
            nc.vector.memset(ones[:], 1.0)
            for b in range(nblk):
                nc.tensor.matmul(out=psum_pool[:, :H], lhsT=Mm[:, b * GPC:(b + 1) * GPC],
                                 rhs=h[:, b, :], start=(b == 0), stop=(b == nblk - 1),
                                 skip_group_check=True)
                nc.tensor.matmul(out=psum_pool[:, H:H + 1], lhsT=Mm[:, b * GPC:(b + 1) * GPC],
                                 rhs=ones[:], start=(b == 0), stop=(b == nblk - 1),
                                 skip_group_check=True)
            psum = sb.tile([GPC, H], f32)
            nc.vector.tensor_copy(psum[:], psum_pool[:, :H])
            cinv = sb.tile([GPC, 1], f32)
            nc.vector.reciprocal(cinv[:], cntt[:])
            pmean = sb.tile([GPC, H], f32)
            nc.vector.tensor_scalar(out=pmean[:], in0=psum[:], scalar1=cinv[:, :1],
                                    scalar2=None, op0=Alu.mult)
            # max pool: per-graph reduce over hT free ranges
            mxT = sb.tile([P, GPC], f32)
            hT_flat = hT[:].rearrange("p b f -> p (b f)")
            for g in range(GPC):
                nc.vector.tensor_reduce(
                    out=mxT[:, g:g + 1],
                    in_=hT_flat[:, g * gpad:(g + 1) * gpad],
                    axis=mybir.AxisListType.X, op=Alu.max)
            # transpose mean/sum -> [128, GPC]
            identg = ident[:GPC, :GPC]
            meanT_p = ps_tr.tile([P, P], f32, space="PSUM", tag="tr")
            nc.tensor.transpose(out=meanT_p[:H, :GPC], in_=pmean[:], identity=identg)
            meanT = sb.tile([P, GPC], f32)
            nc.scalar.copy(meanT[:H, :], meanT_p[:H, :GPC])
            sumT_p = ps_tr.tile([P, P], f32, space="PSUM", tag="tr")
            nc.tensor.transpose(out=sumT_p[:H, :GPC], in_=psum[:], identity=identg)
            sumT = sb.tile([P, GPC], f32)
            nc.scalar.copy(sumT[:H, :], sumT_p[:H, :GPC])

            # ================= global-feature MLP (feature-major) ==========
            r1p = ps_seg.tile([H // 2, GPC], f32, space="PSUM", tag="acc")
            nc.tensor.matmul(out=r1p[:], lhsT=gpW1[:], rhs=gfT[:], start=True, stop=True)
            r1 = sb.tile([H // 2, GPC], f32)
            nc.scalar.activation(r1[:], r1p[:], Act.Relu, bias=gpb1[:, :1], scale=1.0)
            gep = ps_seg.tile([H, GPC], f32, space="PSUM", tag="acc")
            nc.tensor.matmul(out=gep[:], lhsT=gpW2[:], rhs=r1[:], start=True, stop=True)
            geT = sb.tile([H, GPC], f32)
            nc.scalar.activation(geT[:], gep[:], Act.Identity, bias=gpb2[:, :1], scale=1.0)

            # ================= decoder =================
            z1p = ps_pool.tile([GPC, 2 * H], f32, space="PSUM", tag="dec")
            for i, kt in enumerate([meanT, mxT, sumT, geT]):
                nc.tensor.matmul(out=z1p[:], lhsT=kt[:, :GPC] if i != 3 else kt[:],
                                 rhs=dW1[:, i * 2 * H:(i + 1) * 2 * H],
                                 start=(i == 0), stop=(i == 3), skip_group_check=True)
            # LN(z1) graph-major [GPC, 256] + relu
            z1 = sb.tile([GPC, 2 * H], f32)
            nc.vector.tensor_tensor(out=z1[:], in0=z1p[:], in1=db1[:], op=Alu.add)
            nsum = sb.tile([GPC, 1], f32)
            nc.vector.tensor_reduce(out=nsum[:], in_=z1[:], axis=mybir.AxisListType.X,
                                    op=Alu.add, negate=True)
            nmu = sb.tile([GPC, 1], f32)
            nc.scalar.mul(nmu[:], nsum[:], 1.0 / (2 * H))
            zc = sb.tile([GPC, 2 * H], f32)
            nc.scalar.activation(zc[:], z1[:], Act.Identity, bias=nmu[:, :1], scale=1.0)
            sq = sb.tile([GPC, 2 * H], f32)
            ssd = sb.tile([GPC, 1], f32)
            nc.scalar.activation(sq[:], zc[:], Act.Square, accum_out=ssd[:, :1])
            rstd = sb.tile([GPC, 1], f32)
            nc.scalar.activation(rstd[:], ssd[:], Act.Sqrt, bias=eps_t[:GPC, :1], scale=1.0 / (2 * H))
            nc.vector.reciprocal(rstd[:], rstd[:])
            nc.vector.tensor_scalar(out=zc[:], in0=zc[:], scalar1=rstd[:, :1],
                                    scalar2=None, op0=Alu.mult)
            nc.vector.tensor_tensor(out=zc[:], in0=zc[:], in1=dg1[:], op=Alu.mult)
            nc.vector.tensor_tensor(out=zc[:], in0=zc[:], in1=dbe1[:], op=Alu.add)
            nc.vector.tensor_scalar(out=zc[:], in0=zc[:], scalar1=0.0, scalar2=None,
                                    op0=Alu.max)
            # z2 = relu(z1n @ dW2): need z1n^T k-tiles [128, GPC] x2
            z2p_t = ps_pool.tile([GPC, 2 * H], f32, space="PSUM", tag="dec")
            z2p = z2p_t[:, :H]
            for i in range(2):
                ztp = ps_tr.tile([P, P], f32, space="PSUM", tag="tr")
                nc.tensor.transpose(out=ztp[:, :GPC], in_=zc[:, i * P:(i + 1) * P],
                                    identity=identg)
                zt = sb.tile([P, GPC], f32, tag="zt")
                nc.scalar.copy(zt[:], ztp[:, :GPC])
                nc.tensor.matmul(out=z2p[:], lhsT=zt[:], rhs=dW2[:, i * H:(i + 1) * H],
                                 start=(i == 0), stop=(i == 1), skip_group_check=True)
            z2 = sb.tile([GPC, H], f32)
            nc.vector.tensor_tensor(out=z2[:], in0=z2p[:], in1=db2[:], op=Alu.add)
            nc.vector.tensor_scalar(out=z2[:], in0=z2[:], scalar1=0.0, scalar2=None,
                                    op0=Alu.max)
            # z3 = z2 @ dW3 + b3 -> softmax
            z2tp = ps_tr.tile([P, P], f32, space="PSUM", tag="tr")
            nc.tensor.transpose(out=z2tp[:, :GPC], in_=z2[:], identity=identg)
            z2t = sb.tile([P, GPC], f32)
            nc.scalar.copy(z2t[:], z2tp[:, :GPC])
            z3p_t = ps_pool.tile([GPC, 2 * H], f32, space="PSUM", tag="dec")
            z3p = z3p_t[:, :OUT]
            nc.tensor.matmul(out=z3p[:], lhsT=z2t[:], rhs=dW3[:], start=True, stop=True)
            z3 = sb.tile([GPC, OUT], f32)
            nc.vector.tensor_tensor(out=z3[:], in0=z3p[:], in1=db3[:], op=Alu.add)
            nmx = sb.tile([GPC, 1], f32)
            nc.vector.tensor_reduce(out=nmx[:], in_=z3[:], axis=mybir.AxisListType.X,
                                    op=Alu.max, negate=True)
            ez = sb.tile([GPC, OUT], f32)
            nc.scalar.activation(ez[:], z3[:], Act.Exp, bias=nmx[:, :1], scale=1.0)
            sez = sb.tile([GPC, 1], f32)
            nc.vector.tensor_reduce(out=sez[:], in_=ez[:], axis=mybir.AxisListType.X,
                                    op=Alu.add)
            nc.vector.reciprocal(sez[:], sez[:])
            res = sb.tile([GPC, OUT], f32)
            nc.vector.tensor_scalar(out=res[:], in0=ez[:], scalar1=sez[:, :1],
                                    scalar2=None, op0=Alu.mult)
            nc.sync.dma_start(out_d[:], res[:])
    nc.finalize()
    return nc


_CACHE = {}


def kernel(**inputs):
    x = np.asarray(inputs["x"], np.float32)
    edge_index = np.asarray(inputs["edge_index"], np.int32)
    batch = np.asarray(inputs["batch"], np.int32)
    gpad, npad, nblk, nwin, t_w, ntile, cores = _prep(x, edge_index, batch)

    key = (gpad, t_w)
    if key not in _CACHE:
        _CACHE[key] = _build(npad, nblk, nwin, t_w, ntile, gpad)
    nc = _CACHE[key]

    def rep(v, rows):  # replicate 1-D param across partitions
        return np.tile(np.asarray(v, np.float32)[None, :], (rows, 1))

    convW = np.asarray(inputs["conv_W"], np.float32)      # [L, H, H]
    shared = dict(
        encW=np.asarray(inputs["enc_W"], np.float32),
        encb=rep(inputs["enc_b"], P), encg=rep(inputs["enc_gamma"], P),
        encbe=rep(inputs["enc_beta"], P),
        convW=np.concatenate([convW[l] for l in range(L)], axis=1),
        convb=np.concatenate([rep(inputs["conv_b"][l], P) for l in range(L)], 1),
        cg=np.concatenate([rep(inputs["norm_gamma"][l], P) for l in range(L)], 1),
        cb=np.concatenate([rep(inputs["norm_beta"][l], P) for l in range(L)], 1),
        gpW1=np.asarray(inputs["gp_W1"], np.float32),
        gpb1=np.asarray(inputs["gp_b1"], np.float32)[:, None],
        gpW2=np.asarray(inputs["gp_W2"], np.float32),
        gpb2=np.asarray(inputs["gp_b2"], np.float32)[:, None],
        dW1=np.concatenate([np.asarray(inputs["dec_W1"], np.float32)[i * P:(i + 1) * P, :]
                            for i in range(4)], axis=1),
        db1=rep(inputs["dec_b1"], GPC), dg1=rep(inputs["dec_gamma"], GPC),
        dbe1=rep(inputs["dec_beta"], GPC),
        dW2=np.concatenate([np.asarray(inputs["dec_W2"], np.float32)[i * P:(i + 1) * P, :]
                            for i in range(2)], axis=1),
        db2=rep(inputs["dec_b2"], GPC),
        dW3=np.asarray(inputs["dec_W3"], np.float32),
        db3=rep(inputs["dec_b3"], GPC),
    )
    gfull = np.asarray(inputs["global_features"], np.float32)
    in_maps = []
    for c in range(NCORES):
        m = dict(shared)
        m.update(idx=cores[c]["idx"], S=cores[c]["S"], xT=cores[c]["xT"],
                 deg=cores[c]["deg"], M=cores[c]["M"], cnt=cores[c]["cnt"],
                 gfT=gfull[c * GPC:(c + 1) * GPC].T.copy())
        in_maps.append({k: np.ascontiguousarray(v, np.float32) if k != "idx"
                        else np.ascontiguousarray(v, np.int32) for k, v in m.items()})

    from concourse.bass_utils import run_bass_kernel_spmd
    global LAST_NC, LAST_IN_MAPS, LAST_EXEC_NS
    LAST_NC, LAST_IN_MAPS = nc, in_maps
    res = run_bass_kernel_spmd(nc, in_maps, core_ids=list(range(NCORES)))
    LAST_EXEC_NS = res.exec_time_ns
    out = np.concatenate([res.results[c]["out"] for c in range(NCORES)], axis=0)
    return out.astype(np.float32)
